# revision 30
# baseline (speedup 1.0000x reference)
"""Trainium2 Bass kernel for an ODE-RNN encoder (z0 posterior).

Model: 128-step reversed-time GRU-like recurrence with an Euler ODE step on
the mean channel, then a final transform producing (mean_z0, std_z0).

Strategy: data-parallel over the subject (batch) dim across 8 NeuronCores,
weights replicated.  Everything runs on-chip in a transposed layout
([feature, batch], batch=256 on the free dim).  v3 rework versus v2
(1.49 ms): the PE is issue-bound (~213 ns per weight-switching matmul of
free=256, vs 107 ns of array time) and idles ~4 us/step waiting on the
ACT/DVE recurrence chain, so v3 removes matmul slots and fills the chain
stalls:
- All K=1 bias-row matmuls are gone: ode1_b1 rides the h_ode tanh as a
  per-half ACT bias column; ode_b2 and the (uniform) Euler dt are folded
  into host-prescaled ode2 weights plus a b2*dt bias column consumed by
  the yode DVE op; tz1_b rides the final h_tz tanh the same way.
- t=0 is special-cased: state is zero, so yode(0) = dt0*ode_f(0) is a
  host-precomputed constant column (one DVE op) and the k1@ys / k1@as
  matmuls (ys=0) are skipped.
- psAr/psAu/psC are double-buffered (2 PSUM banks each) and psF/psD/psE
  share one bank F->D->E (their lifetimes are sequential within a step),
  for exactly 8 banks.  This lets the x-part (kx) matmuls of step t+1 run
  inside step t's two big PE stall windows (the h_ode tanh and the yode
  DVE op); x/mask DMAs are issued two steps ahead to cover them.
- u-gate sign flip: t_ur_u = tanh(-z/2 - b/2) makes
  G = m*(1-sigmoid(z)) = (t_ur_u + 1) * (m/2) one DVE op (the mask DMA
  carries 0.5*m), and T1 = (ln2*G)*u folds the ln2 scale into one stt;
  q_u/gl of v2 are gone.
- sigmoid via tanh and softplus(x) = relu(x) + ln2*u + (1+u)*2^{-u}
  (u = e^{-|x|}, exact one-Newton log1p(e^x)) keep every transcendental
  in the resident `exp_and_others` ACT table set; slack-tolerant softplus
  ops run on the otherwise idle GPSIMD/Pool engine.
- matmul operands and the recurrent state are bf16 (fp32 PSUM
  accumulate): fp32 matmuls lower to TWO half-speed PE passes, bf16 is
  single-pass with fast weight load.
"""
import sys
import numpy as np
import ml_dtypes

for _p in ("/opt/trn_rl_repo", "/root/.axon_site/_ro/trn_rl_repo"):
    if _p not in sys.path:
        sys.path.append(_p)

N_SUBJ, N_TP, INPUT_DIM, LATENT, N_UNIT = 2048, 128, 64, 128, 256
HALF = INPUT_DIM // 2
N_CORES = 8
B = N_SUBJ // N_CORES          # 256 subjects per core (free dim)
L = LATENT
LN2 = float(np.log(2.0))
BF = ml_dtypes.bfloat16

_CACHE = {}


# --------------------------------------------------------------------------
# Bass program
# --------------------------------------------------------------------------
def _build(n_tp, dts):
    import concourse.mybir as mybir
    from concourse import bacc, tile

    F32 = mybir.dt.float32
    B16 = mybir.dt.bfloat16
    AF = mybir.ActivationFunctionType
    OP = mybir.AluOpType

    # dt is uniform for t>=1 (obs_tps is a linspace); t=0 is special-cased
    # with a host-precomputed constant, so the Euler dt can be pre-folded
    # into the ode2 weights.
    assert n_tp >= 3
    assert all(abs(dts[i] - dts[1]) < 1e-5 for i in range(1, n_tp)), dts

    # Bacc (not plain Bass): its compile() legalizes the TRN2 one-sync-wait-
    # per-instruction limit (event-semaphore splitting, matmul-wait moves).
    nc = bacc.Bacc(None)

    # ---- DRAM I/O ----
    d_x = nc.dram_tensor("x_rev", [n_tp, INPUT_DIM, B], B16, kind="ExternalInput")
    d_m = nc.dram_tensor("m_bc", [n_tp, 128, B], B16, kind="ExternalInput")

    bspec = {  # bf16 weights (matmul operands)
        "ug1_k0": [L, N_UNIT], "ug1_k1": [L, N_UNIT], "ug1_kx": [INPUT_DIM + 1, N_UNIT],
        "rg1_k0": [L, N_UNIT], "rg1_k1": [L, N_UNIT], "rg1_kx": [INPUT_DIM + 1, N_UNIT],
        "ns1_k0": [L, N_UNIT], "ns1_k1": [L, N_UNIT], "ns1_kx": [INPUT_DIM + 1, N_UNIT],
        "ode1_w": [L, N_UNIT],
        "neg_eye": [L, L],
        "ode2_k0": [128, L], "ode2_k1": [128, L],
        "ug2_k0": [128, L], "ug2_k1": [128, L],
        "rg2_k0": [128, L], "rg2_k1": [128, L],
        "ns2_k0": [128, 2 * L], "ns2_k1": [128, 2 * L],
        "tz1_k0": [L, N_UNIT], "tz1_k1": [L, N_UNIT],
        "tz2_k0": [128, 2 * L], "tz2_k1": [128, 2 * L],
    }
    fspec = {  # fp32 per-partition columns (ACT bias / DVE scalar APs)
        "ug2_bc": [128, 1], "rg2_bc": [128, 1],
        "ns2_bm": [128, 1], "ns2_bs": [128, 1], "tz2_bm": [128, 1], "tz2_bs": [128, 1],
        "ode1_b1c0": [128, 1], "ode1_b1c1": [128, 1], "ode_b2c": [128, 1],
        "tz1_bc0": [128, 1], "tz1_bc1": [128, 1], "yode0_c": [128, 1],
    }
    d_w = {k: nc.dram_tensor(k, v, B16, kind="ExternalInput") for k, v in bspec.items()}
    d_w.update({k: nc.dram_tensor(k, v, F32, kind="ExternalInput")
                for k, v in fspec.items()})

    d_om = nc.dram_tensor("out_m", [L, B], F32, kind="ExternalOutput")
    d_os = nc.dram_tensor("out_s", [L, B], F32, kind="ExternalOutput")

    CC = float(np.float32(1e-6) - np.float32(1.0))

    with tile.TileContext(nc) as tc:
        with (
            tc.tile_pool(name="const", bufs=1) as cp,
            tc.tile_pool(name="work", bufs=3) as wp,
            tc.tile_pool(name="ps", bufs=1, space="PSUM") as pp,
            tc.tile_pool(name="ps2", bufs=2, space="PSUM") as pp2,
        ):
            # ---- resident constants / weights ----
            w = {}
            for k, shp in bspec.items():
                w[k] = cp.tile(shp, B16, tag=k, name=k)
                nc.sync.dma_start(w[k][:], d_w[k][:])
            for k, shp in fspec.items():
                w[k] = cp.tile(shp, F32, tag=k, name=k)
                nc.sync.dma_start(w[k][:], d_w[k][:])

            xbufs = []
            for j in range(3):
                xb = cp.tile([INPUT_DIM + 1, B], B16, tag=f"xb{j}", name=f"xb{j}")
                nc.vector.memset(xb[INPUT_DIM:, :], 1.0)
                xbufs.append(xb)
            mbufs = [cp.tile([128, B], B16, tag=f"mb{j}", name=f"mb{j}")
                     for j in range(3)]

            # state lives in bf16 (the bf16 matmul-input rounding dominates)
            ym = [cp.tile([L, B], B16, tag=f"ym{i}", name=f"ym{i}") for i in range(2)]
            ys = [cp.tile([L, B], B16, tag=f"ys{i}", name=f"ys{i}") for i in range(2)]
            nc.vector.memset(ys[0][:], 0.0)

            mm = nc.tensor.matmul

            # Warm the PE's clock past every weight DMA with K=1 dummy
            # matmuls so steady-state matmuls only wait on one producer.
            psDEF = pp.tile([128, 2 * B], F32, tag="psDEF", name="psDEF_warm")
            for k in bspec:
                mm(psDEF[0:1, 0:1], w[k][0:1, 0:1], w[k][0:1, 1:2],
                   start=True, stop=True)
            # DVE/ACT read fp32 DMA-produced columns: warm those clocks too
            nf = len(fspec)
            warm_dv = cp.tile([1, 2 * nf], F32, tag="warm_dv", name="warm_dv")
            for j, k in enumerate(fspec):
                nc.vector.tensor_copy(warm_dv[0:1, j:j + 1], w[k][0:1, 0:1])
                nc.scalar.copy(warm_dv[0:1, nf + j:nf + j + 1], w[k][0:1, 0:1])

            # ---- the recurrence ----
            # Emission order == scheduler priority: within each engine, ops
            # are emitted in intended steady-state execution order.  The
            # kx (x-part) matmuls of step t+1 are emitted inside step t's
            # two PE stall windows (h_ode tanh, yode DVE) -- psAr/psAu/psC
            # are double-buffered so the target bank is free by then.
            from concourse.tile_rust import add_dep_helper
            prev_tail = None
            psB = None          # ode1 hidden pre-act of the CURRENT step
            psA = {}            # psAr/psAu/psC of the current step
            psA_n = {}          # ... of the next step (kx pre-accumulated)

            def kx_mms(t, nets):
                """x-part matmuls of step t into fresh double-buffered banks."""
                xbt = xbufs[t % 3]
                out = {}
                for net in nets:
                    ps = pp2.tile([128, 2 * B], F32, tag="ps" + net[0].upper(),
                                  name=f"ps{net[0]}{t}")
                    for m in range(2):
                        ms = slice(m * 128, (m + 1) * 128)
                        mm(ps[:, m * B:(m + 1) * B], w[net + "_kx"][:, ms],
                           xbt[:], start=(m == 0), stop=False)
                    out["ps" + net[0].upper()] = ps
                return out

            for t in range(n_tp):
                first, last = t == 0, t == n_tp - 1
                cur, nxt = t % 2, (t + 1) % 2
                mb = mbufs[t % 3]

                if first:
                    nc.sync.dma_start(xbufs[0][:INPUT_DIM, :], d_x[0])
                    nc.sync.dma_start(mbufs[0][:], d_m[0])
                    nc.sync.dma_start(xbufs[1][:INPUT_DIM, :], d_x[1])
                    nc.sync.dma_start(mbufs[1][:], d_m[1])
                    psA = kx_mms(0, ("rg1", "ug1", "ns1"))
                if t + 2 < n_tp:
                    nc.sync.dma_start(xbufs[(t + 2) % 3][:INPUT_DIM, :], d_x[t + 2])
                    nc.sync.dma_start(mbufs[(t + 2) % 3][:], d_m[t + 2])

                psAr, psAu, psC = psA["psR"], psA["psU"], psA["psN"]

                # ---- ODE update ----
                yode = wp.tile([L, B], B16, tag="yode", name="yode")
                if first:
                    # state is zero: yode(0) = dt0*ode_f(0), a host constant
                    nc.vector.tensor_scalar(yode[:], ys[0][:],
                                            w["yode0_c"][:, 0:1], None, op0=OP.add)
                    psDEF_t = psDEF
                    psA_n = kx_mms(1, ("rg1", "ug1", "ns1"))
                else:
                    psDEF_t = pp.tile([128, 2 * B], F32, tag="psDEF",
                                      name=f"psDEF{t}")
                    psF = psDEF_t[:, 0:B]
                    # h_ode halves carry ode1_b1 as ACT bias; ode2 weights are
                    # dt-prescaled so yode needs no extra multiply
                    h_ode = wp.tile([128, 2 * B], B16, tag="h_ode", name="h_ode")
                    nc.scalar.activation(h_ode[:, 0:B], psB[:, 0:B], AF.Tanh,
                                         bias=w["ode1_b1c0"][:, 0:1])
                    # stall window W0 (h_ode tanh): kx of step t+1, part 1
                    if not last:
                        psA_n = kx_mms(t + 1, ("rg1", "ug1"))
                    mm(psF, w["ode2_k0"][:], h_ode[:, 0:B],
                       start=True, stop=False)
                    nc.scalar.activation(h_ode[:, B:], psB[:, B:], AF.Tanh,
                                         bias=w["ode1_b1c1"][:, 0:1])
                    mm(psF, w["ode2_k1"][:], h_ode[:, B:], start=False, stop=True)
                    # Yode = (psF + dt*b2) + Ym: ONE DVE op
                    nc.vector.scalar_tensor_tensor(yode[:], psF, w["ode_b2c"][:, 0:1],
                                                   ym[cur][:], op0=OP.add, op1=OP.add)
                    # stall window W1 (yode DVE): kx of step t+1, part 2
                    if not last:
                        psA_n.update(kx_mms(t + 1, ("ns1",)))

                # deferred ys' of the previous step: emitted HERE so it sits
                # after yode in the DVE static order
                if prev_tail is not None:
                    pPpre, pT1, pQ = prev_tail
                    A2 = wp.tile([L, B], F32, tag="A2", name="A2")
                    nc.vector.tensor_tensor(A2[:], pPpre[:], pT1[:], op=OP.add)
                    nc.vector.tensor_tensor(ys[cur][:], A2[:], pQ[:], op=OP.add)
                    prev_tail = None

                # gate layer 1 state parts.  PE order is chosen so psAr m0
                # (the r-chain) completes earliest: rg_k1 m0 sits right where
                # the ys' adds land (~yode+1.3us), with ug/ode1 matmuls
                # filling the gap before it.
                if not last:
                    psBn = pp.tile([128, 2 * B], F32, tag="psB", name=f"psBn{t}")

                def l1(net, ps, kk, m, stop=False):
                    ms = slice(m * 128, (m + 1) * 128)
                    src = yode if kk == "_k0" else ys[cur]
                    return mm(ps[:, m * B:(m + 1) * B], w[net + kk][:, ms], src[:],
                              start=False, stop=stop)

                def ode1(m, src):
                    ms = slice(m * 128, (m + 1) * 128)
                    return mm(psBn[:, m * B:(m + 1) * B], w["ode1_w"][:, ms],
                              src[:], start=(m == 0), stop=False)

                l1("rg1", psAr, "_k0", 0)
                l1("ug1", psAu, "_k0", 0)
                l1("ug1", psAu, "_k0", 1, stop=first)
                l1("rg1", psAr, "_k0", 1, stop=first)
                i_k1 = None
                if not first:  # ys(t=0) == 0: skip the k1 matmuls entirely
                    l1("rg1", psAr, "_k1", 0)
                    l1("rg1", psAr, "_k1", 1, stop=True)
                    l1("ug1", psAu, "_k1", 0)
                    i_k1 = l1("ug1", psAu, "_k1", 1, stop=True)
                if not first:
                    # ns1's ys part, split off the r-gate: 0.5*w1*(1+t_r)*ys
                    # == k1@ys (here, hidden) + k1@(t_r*ys) (late) with the
                    # 0.5 in the weights -- removes the as stt from the
                    # h_ns-gating path
                    for m in range(2):
                        ms = slice(m * 128, (m + 1) * 128)
                        mm(psC[:, m * B:(m + 1) * B], w["ns1_k1"][:, ms],
                           ys[cur][:], start=False, stop=False)
                # ode1@yode is off-cycle (psBn only closes at pm-time): FORCE
                # it after the tanh-gating k1 matmuls (the scheduler otherwise
                # hoists it, delaying tanh_r by two matmul slots)
                if not last:
                    i_o0 = ode1(0, yode)
                    if i_k1 is not None:
                        add_dep_helper(i_o0.ins, i_k1.ins, False, "pe-order")
                    ode1(1, yode)

                # r-gate layer 2 with m-half wavefront; psD shares the
                # psDEF bank (F is consumed by yode before D is written)
                psD_u, psD_r = psDEF_t[:, 0:B], psDEF_t[:, B:]
                h_r = wp.tile([128, 2 * B], B16, tag="h_r", name="h_r")
                t_ur = wp.tile([128, 2 * B], B16, tag="t_ur", name="t_ur")
                i_hr1 = None
                for m in range(2):
                    i_hr1 = nc.scalar.activation(h_r[:, m * B:(m + 1) * B],
                                                 psAr[:, m * B:(m + 1) * B], AF.Tanh)
                    mm(psD_r, w[f"rg2_k{m}"][:], h_r[:, m * B:(m + 1) * B],
                       start=(m == 0), stop=(m == 1))
                nc.scalar.activation(t_ur[:, B:], psD_r, AF.Tanh,
                                     bias=w["rg2_bc"][:, 0:1], scale=0.5)
                h_u = wp.tile([128, 2 * B], B16, tag="h_u", name="h_u")
                i_hu = nc.scalar.activation(h_u[:], psAu[:], AF.Tanh)
                # ACT order: the off-cycle merged h_u must not run between
                # the two h_r halves (the r path is the critical cycle)
                add_dep_helper(i_hu.ins, i_hr1.ins, False, "act-order")
                for m in range(2):
                    mm(psD_u, w[f"ug2_k{m}"][:], h_u[:, m * B:(m + 1) * B],
                       start=False, stop=(m == 1))
                # sign-flipped u half: t_ur_u = tanh(-z/2 - b/2), so
                # G = m*(1-sigmoid(z)) = (t_ur_u + 1) * (m/2)
                nc.scalar.activation(t_ur[:, 0:B], psD_u, AF.Tanh,
                                     bias=w["ug2_bc"][:, 0:1], scale=-0.5)

                # reset products via the prescale trick: ns1_k0/k1 carry a
                # host-side 0.5 factor, so r.state == 0.5*(1+tanh)*state
                am = wp.tile([L, B], B16, tag="am", name="am")
                nc.vector.scalar_tensor_tensor(am[:], t_ur[:, B:], 1.0, yode[:],
                                               op0=OP.add, op1=OP.mult)
                a_s = None
                if not first:
                    a_s = wp.tile([L, B], B16, tag="a_s", name="a_s")
                    nc.vector.tensor_tensor(a_s[:], t_ur[:, B:], ys[cur][:],
                                            op=OP.mult)
                # G = (t_ur_u + 1) * mh   (m_bc carries 0.5*mask)
                g = wp.tile([L, B], B16, tag="g", name="g")
                nc.vector.scalar_tensor_tensor(g[:], t_ur[:, 0:B], 1.0, mb[:L, :],
                                               op0=OP.add, op1=OP.mult)

                # ns1 state parts, am before as per m-half
                for m in range(2):
                    sl = psC[:, m * B:(m + 1) * B]
                    ms = slice(m * 128, (m + 1) * 128)
                    mm(sl, w["ns1_k0"][:, ms], am[:], start=False,
                       stop=(first and m == 1))
                    if not first:
                        mm(sl, w["ns1_k1"][:, ms], a_s[:], start=False,
                           stop=(m == 1))

                # ns layer 2 with m-half wavefront; NM first (mean cycle),
                # neg_eye (-Yode, ready early) fills the h_ns-m1 gap.
                # psE shares the psDEF bank (D consumed by t_ur by now).
                psE_m, psE_s = psDEF_t[:, 0:B], psDEF_t[:, B:]
                h_ns = wp.tile([128, 2 * B], B16, tag="h_ns", name="h_ns")
                nc.scalar.activation(h_ns[:, 0:B], psC[:, 0:B], AF.Tanh)
                i_nm0 = mm(psE_m, w["ns2_k0"][:, 0:128], h_ns[:, 0:B],
                           start=True, stop=False)
                i_ne = mm(psE_m, w["neg_eye"][:], yode[:], start=False, stop=False)
                add_dep_helper(i_ne.ins, i_nm0.ins, False, "bank-start")
                nc.scalar.activation(h_ns[:, B:], psC[:, B:], AF.Tanh)
                mm(psE_m, w["ns2_k1"][:, 0:128], h_ns[:, B:],
                   start=False, stop=False)
                mm(psE_s, w["ns2_k0"][:, 128:], h_ns[:, 0:B],
                   start=False, stop=False)
                mm(psE_s, w["ns2_k1"][:, 128:], h_ns[:, B:],
                   start=False, stop=True)

                # mean channel: Ym' = Yode + G*(NM + bm - Yode); psE_m
                # already holds NM - Yode via neg_eye, so ONE stt + add
                pm = wp.tile([L, B], B16, tag="pm", name="pm")
                nc.vector.scalar_tensor_tensor(
                    pm[:], psE_m, w["ns2_bm"][:, 0:1], g[:],
                    op0=OP.add, op1=OP.mult)
                # Ym' rides the idle Pool engine: its only consumer is the
                # NEXT step's yode op, a full step away
                nc.gpsimd.tensor_tensor(ym[nxt][:], yode[:], pm[:], op=OP.add)
                if not last:
                    for m in range(2):
                        ms = slice(m * 128, (m + 1) * 128)
                        mm(psBn[:, m * B:(m + 1) * B], w["ode1_w"][:, ms], pm[:],
                           start=False, stop=(m == 1))
                    psB = psBn
                    psA = psA_n

                # std channel: sp(z)+1e-6 = relu(z) + ln2*u + (1+u)*2^{-u}
                # + (1e-6 - 1),  u = e^{-|z|}  (exact one-Newton log1p(e^z)).
                # Ys' = (P0 + T1) + Q: P0 = Ys + G*(rl + c - Ys) via Pool,
                # T1 = (ln2*G)*u, Q = (G*(1+u))*v, v = 2^{-u}; only Q and
                # two adds trail the exps.
                # DVE P0 path emitted BEFORE the ACT chain so rl's wait
                # anchors to the ns2 matmul sem directly (emitted after zb,
                # the legalizer re-anchors it to zb's ACT sem, ~+800ns, and
                # the whole P0a->Ppre->A2 chain slides past yode)
                rl = wp.tile([L, B], F32, tag="rl", name="rl")
                nc.vector.tensor_scalar(rl[:], psE_s, w["ns2_bs"][:, 0:1],
                                        0.0, op0=OP.add, op1=OP.max)
                sB = wp.tile([L, B], F32, tag="sB", name="sB")
                nc.vector.scalar_tensor_tensor(sB[:], rl[:], CC, ys[cur][:],
                                               op0=OP.add, op1=OP.subtract)
                # slack-tolerant fp32 product on the otherwise idle Pool
                P0a = wp.tile([L, B], F32, tag="P0a", name="P0a")
                nc.gpsimd.tensor_tensor(P0a[:], sB[:], g[:], op=OP.mult)
                zb = wp.tile([L, B], F32, tag="zb", name="zb")
                nc.scalar.activation(zb[:], psE_s, AF.Abs,
                                     bias=w["ns2_bs"][:, 0:1])
                u_e = wp.tile([L, B], B16, tag="u_e", name="u_e")
                nc.scalar.activation(u_e[:], zb[:], AF.Exp, scale=-1.0)
                v_e = wp.tile([L, B], B16, tag="v_e", name="v_e")
                nc.scalar.activation(v_e[:], u_e[:], AF.Exp, scale=-LN2)
                # ug = u*G, then T1 = ln2*ug and gw = G + ug = (1+u)*G.
                # (The one-stt (u+1)*G form measures 891ns -- the add+mult
                # stt has no 2x uop -- so TT/TS ops are faster AND unblock
                # the next yode in the DVE queue.)
                ug = wp.tile([L, B], B16, tag="ug", name="ug")
                nc.vector.tensor_tensor(ug[:], u_e[:], g[:], op=OP.mult)
                T1 = wp.tile([L, B], B16, tag="T1", name="T1")
                nc.vector.tensor_scalar(T1[:], ug[:], LN2, None, op0=OP.mult)
                gw = wp.tile([L, B], B16, tag="gw", name="gw")
                nc.vector.tensor_tensor(gw[:], g[:], ug[:], op=OP.add)
                # Q also rides the Pool (deadline: the ys' add early next step)
                Q_ = wp.tile([L, B], B16, tag="Q_", name="Q_")
                nc.gpsimd.tensor_tensor(Q_[:], gw[:], v_e[:], op=OP.mult)
                Ppre = wp.tile([L, B], F32, tag="Ppre", name="Ppre")
                nc.vector.tensor_tensor(Ppre[:], ys[cur][:], P0a[:], op=OP.add)
                prev_tail = (Ppre, T1, Q_)

            # ---- final transform ----
            fin = n_tp % 2
            if prev_tail is not None:
                pPpre, pT1, pQ = prev_tail
                A2 = wp.tile([L, B], F32, tag="A2", name="A2fin")
                nc.vector.tensor_tensor(A2[:], pPpre[:], pT1[:], op=OP.add)
                nc.vector.tensor_tensor(ys[fin][:], A2[:], pQ[:], op=OP.add)
            psB = pp.tile([128, 2 * B], F32, tag="psB", name="psB_fin")
            for m in range(2):
                sl = psB[:, m * B:(m + 1) * B]
                ms = slice(m * 128, (m + 1) * 128)
                mm(sl, w["tz1_k0"][:, ms], ym[fin][:], start=(m == 0), stop=False)
                mm(sl, w["tz1_k1"][:, ms], ys[fin][:], start=False, stop=True)
            h_tz = wp.tile([128, 2 * B], B16, tag="h_ode", name="h_tz")
            nc.scalar.activation(h_tz[:, 0:B], psB[:, 0:B], AF.Tanh,
                                 bias=w["tz1_bc0"][:, 0:1])
            nc.scalar.activation(h_tz[:, B:], psB[:, B:], AF.Tanh,
                                 bias=w["tz1_bc1"][:, 0:1])
            psE = pp.tile([128, 2 * B], F32, tag="psDEF", name="psDEF_fin")
            for m in range(2):
                sl = psE[:, m * B:(m + 1) * B]
                ms = slice(m * 128, (m + 1) * 128)
                mm(sl, w["tz2_k0"][:, ms], h_tz[:, 0:B], start=(m == 0), stop=False)
                mm(sl, w["tz2_k1"][:, ms], h_tz[:, B:], start=False, stop=True)
            o_m = wp.tile([L, B], F32, tag="o_m", name="o_m")
            nc.scalar.activation(o_m[:], psE[:, 0:B], AF.Identity,
                                 bias=w["tz2_bm"][:, 0:1])
            o_s = wp.tile([L, B], F32, tag="o_s", name="o_s")
            nc.scalar.activation(o_s[:], psE[:, B:], AF.Abs,
                                 bias=w["tz2_bs"][:, 0:1])
            nc.sync.dma_start(d_om[:], o_m[:])
            nc.sync.dma_start(d_os[:], o_s[:])

    nc.compile()
    return nc


# --------------------------------------------------------------------------
# host-side packing
# --------------------------------------------------------------------------
def _dts(obs, n_tp):
    F = np.float32
    dd = (obs[:-1] - obs[1:])[::-1]
    return np.concatenate([np.full((1,), -0.01, F), dd]).astype(F)


def _prep_in_maps(inputs, n_tp):
    F = np.float32
    d = {k: np.ascontiguousarray(np.asarray(v, F)) for k, v in inputs.items()}
    data = d["data"][:, :n_tp]
    obs = np.asarray(inputs["obs_tps"], F)[:n_tp]
    dts = _dts(obs, n_tp)
    dt1 = F(dts[1])

    # [t, c, subj], reversed in time, bf16
    x_rev = np.ascontiguousarray(data.transpose(1, 2, 0)[::-1]).astype(BF)
    # observation half-mask 0.5*m per (t, subj), broadcast to 128 partitions
    m_t = F(0.5) * (data[:, :, HALF:].sum(axis=2) > 0).astype(F)  # [subj, t]
    m_rev = m_t.T[::-1].astype(BF)                                # [t, subj]
    m_bc = np.ascontiguousarray(
        np.broadcast_to(m_rev[:, None, :], (n_tp, 128, N_SUBJ)))

    def kx(w1, b1):
        return np.vstack([w1[2 * L:], b1[None, :]])

    bf = {
        "ug1_k0": d["ug_w1"][:L], "ug1_k1": d["ug_w1"][L:2 * L],
        "ug1_kx": kx(d["ug_w1"], d["ug_b1"]),
        "rg1_k0": d["rg_w1"][:L], "rg1_k1": d["rg_w1"][L:2 * L],
        "rg1_kx": kx(d["rg_w1"], d["rg_b1"]),
        "ns1_k0": d["ns_w1"][:L] * F(0.5), "ns1_k1": d["ns_w1"][L:2 * L] * F(0.5),
        "ns1_kx": kx(d["ns_w1"], d["ns_b1"]),
        "ode1_w": d["ode_w1"],
        "neg_eye": -np.eye(L, dtype=F),
        "ode2_k0": d["ode_w2"][:128] * dt1, "ode2_k1": d["ode_w2"][128:] * dt1,
        "ug2_k0": d["ug_w2"][:128], "ug2_k1": d["ug_w2"][128:],
        "rg2_k0": d["rg_w2"][:128], "rg2_k1": d["rg_w2"][128:],
        "ns2_k0": d["ns_w2"][:128], "ns2_k1": d["ns_w2"][128:],
        "tz1_k0": d["tz_w1"][:L], "tz1_k1": d["tz_w1"][L:],
        "tz2_k0": d["tz_w2"][:128], "tz2_k1": d["tz_w2"][128:],
    }
    shared = {k: np.ascontiguousarray(v.astype(BF)) for k, v in bf.items()}
    col = lambda v: np.ascontiguousarray(v.reshape(-1, 1).astype(F))
    shared["ug2_bc"] = col(d["ug_b2"] * F(-0.5))   # sign-flipped u half
    shared["rg2_bc"] = col(d["rg_b2"] * F(0.5))
    shared["ns2_bm"] = col(d["ns_b2"][:L])
    shared["ns2_bs"] = col(d["ns_b2"][L:])
    shared["tz2_bm"] = col(d["tz_b2"][:L])
    shared["tz2_bs"] = col(d["tz_b2"][L:])
    shared["ode1_b1c0"] = col(d["ode_b1"][:128])
    shared["ode1_b1c1"] = col(d["ode_b1"][128:])
    shared["ode_b2c"] = col(d["ode_b2"] * dt1)
    shared["tz1_bc0"] = col(d["tz_b1"][:128])
    shared["tz1_bc1"] = col(d["tz_b1"][128:])
    # yode(0) = dt0 * ode_f(0) with zero initial state
    ode_f0 = np.tanh(d["ode_b1"]) @ d["ode_w2"] + d["ode_b2"]
    shared["yode0_c"] = col(ode_f0 * F(dts[0]))

    in_maps = []
    for c in range(N_CORES):
        m = dict(shared)
        m["x_rev"] = np.ascontiguousarray(x_rev[:, :, c * B:(c + 1) * B])
        m["m_bc"] = np.ascontiguousarray(m_bc[:, :, c * B:(c + 1) * B])
        in_maps.append(m)
    return in_maps


def kernel(**inputs):
    from concourse.bass_utils import run_bass_kernel_spmd

    obs = np.asarray(inputs["obs_tps"], np.float32)[:N_TP]
    dts = _dts(obs, N_TP)
    key = (N_TP, tuple(np.asarray(dts, np.float64).tolist()))
    if key not in _CACHE:
        _CACHE[key] = _build(N_TP, dts)
    nc = _CACHE[key]

    in_maps = _prep_in_maps(inputs, N_TP)
    res = run_bass_kernel_spmd(nc, in_maps, list(range(N_CORES)))
    outs = res.results

    mean = np.empty((1, N_SUBJ, L), np.float32)
    std = np.empty((1, N_SUBJ, L), np.float32)
    for c in range(N_CORES):
        mean[0, c * B:(c + 1) * B] = outs[c]["out_m"].T
        std[0, c * B:(c + 1) * B] = outs[c]["out_s"].T
    return mean, std


# revision 31
# speedup vs baseline: 1.2145x; 1.2145x over previous
"""Trainium2 Bass kernel for an ODE-RNN encoder (z0 posterior).

Model: 128-step reversed-time GRU-like recurrence with an Euler ODE step on
the mean channel, then a final transform producing (mean_z0, std_z0).

Strategy: data-parallel over the subject (batch) dim across 8 NeuronCores,
weights replicated.  Everything runs on-chip in a transposed layout
([feature, batch], batch=256 on the free dim).  v3 rework versus v2
(1.49 ms): the PE is issue-bound (~213 ns per weight-switching matmul of
free=256, vs 107 ns of array time) and idles ~4 us/step waiting on the
ACT/DVE recurrence chain, so v3 removes matmul slots and fills the chain
stalls:
- All K=1 bias-row matmuls are gone: ode1_b1 rides the h_ode tanh as a
  per-half ACT bias column; ode_b2 and the (uniform) Euler dt are folded
  into host-prescaled ode2 weights plus a b2*dt bias column consumed by
  the yode DVE op; tz1_b rides the final h_tz tanh the same way.
- t=0 is special-cased: state is zero, so yode(0) = dt0*ode_f(0) is a
  host-precomputed constant column (one DVE op) and the k1@ys / k1@as
  matmuls (ys=0) are skipped.
- psAr/psAu/psC are double-buffered (2 PSUM banks each) and psF/psD/psE
  share one bank F->D->E (their lifetimes are sequential within a step),
  for exactly 8 banks.  This lets the x-part (kx) matmuls of step t+1 run
  inside step t's two big PE stall windows (the h_ode tanh and the yode
  DVE op); x/mask DMAs are issued two steps ahead to cover them.
- u-gate sign flip: t_ur_u = tanh(-z/2 - b/2) makes
  G = m*(1-sigmoid(z)) = (t_ur_u + 1) * (m/2) one DVE op (the mask DMA
  carries 0.5*m), and T1 = (ln2*G)*u folds the ln2 scale into one stt;
  q_u/gl of v2 are gone.
- sigmoid via tanh and softplus(x) = relu(x) + ln2*u + (1+u)*2^{-u}
  (u = e^{-|x|}, exact one-Newton log1p(e^x)) keep every transcendental
  in the resident `exp_and_others` ACT table set; slack-tolerant softplus
  ops run on the otherwise idle GPSIMD/Pool engine.
- matmul operands and the recurrent state are bf16 (fp32 PSUM
  accumulate): fp32 matmuls lower to TWO half-speed PE passes, bf16 is
  single-pass with fast weight load.
"""
import sys
import numpy as np
import ml_dtypes

for _p in ("/opt/trn_rl_repo", "/root/.axon_site/_ro/trn_rl_repo"):
    if _p not in sys.path:
        sys.path.append(_p)

N_SUBJ, N_TP, INPUT_DIM, LATENT, N_UNIT = 2048, 128, 64, 128, 256
HALF = INPUT_DIM // 2
N_CORES = 8
B = N_SUBJ // N_CORES          # 256 subjects per core (free dim)
L = LATENT
LN2 = float(np.log(2.0))
BF = ml_dtypes.bfloat16

_CACHE = {}


# --------------------------------------------------------------------------
# Bass program
# --------------------------------------------------------------------------
def _build(n_tp, dts):
    import concourse.mybir as mybir
    from concourse import bacc, tile

    F32 = mybir.dt.float32
    B16 = mybir.dt.bfloat16
    AF = mybir.ActivationFunctionType
    OP = mybir.AluOpType

    # dt is uniform for t>=1 (obs_tps is a linspace); t=0 is special-cased
    # with a host-precomputed constant, so the Euler dt can be pre-folded
    # into the ode2 weights.
    assert n_tp >= 3
    assert all(abs(dts[i] - dts[1]) < 1e-5 for i in range(1, n_tp)), dts

    # Bacc (not plain Bass): its compile() legalizes the TRN2 one-sync-wait-
    # per-instruction limit (event-semaphore splitting, matmul-wait moves).
    nc = bacc.Bacc(None)

    # ---- DRAM I/O ----
    d_x = nc.dram_tensor("x_rev", [n_tp, INPUT_DIM, B], B16, kind="ExternalInput")
    d_m = nc.dram_tensor("m_bc", [n_tp, 128, B], B16, kind="ExternalInput")

    bspec = {  # bf16 weights (matmul operands)
        "ug1_k0": [L, N_UNIT], "ug1_k1": [L, N_UNIT], "ug1_kx": [INPUT_DIM + 1, N_UNIT],
        "rg1_k0": [L, N_UNIT], "rg1_k1": [L, N_UNIT], "rg1_kx": [INPUT_DIM + 1, N_UNIT],
        "ns1_k0": [L, N_UNIT], "ns1_k1": [L, N_UNIT], "ns1_kx": [INPUT_DIM + 1, N_UNIT],
        "ode1_w": [L, N_UNIT],
        "neg_eye": [L, L],
        "ode2_k0": [128, L], "ode2_k1": [128, L],
        "ug2_k0": [128, L], "ug2_k1": [128, L],
        "rg2_k0": [128, L], "rg2_k1": [128, L],
        "ns2_k0": [128, 2 * L], "ns2_k1": [128, 2 * L],
        "tz1_k0": [L, N_UNIT], "tz1_k1": [L, N_UNIT],
        "tz2_k0": [128, 2 * L], "tz2_k1": [128, 2 * L],
    }
    fspec = {  # fp32 per-partition columns (ACT bias / DVE scalar APs)
        "ug2_bc": [128, 1], "rg2_bc": [128, 1],
        "ns2_bm": [128, 1], "ns2_bs": [128, 1], "tz2_bm": [128, 1], "tz2_bs": [128, 1],
        "ode1_b1c0": [128, 1], "ode1_b1c1": [128, 1], "ode_b2c": [128, 1],
        "tz1_bc0": [128, 1], "tz1_bc1": [128, 1], "yode0_c": [128, 1],
    }
    d_w = {k: nc.dram_tensor(k, v, B16, kind="ExternalInput") for k, v in bspec.items()}
    d_w.update({k: nc.dram_tensor(k, v, F32, kind="ExternalInput")
                for k, v in fspec.items()})

    d_om = nc.dram_tensor("out_m", [L, B], F32, kind="ExternalOutput")
    d_os = nc.dram_tensor("out_s", [L, B], F32, kind="ExternalOutput")

    CC = float(np.float32(1e-6) - np.float32(1.0))

    with tile.TileContext(nc) as tc:
        with (
            tc.tile_pool(name="const", bufs=1) as cp,
            tc.tile_pool(name="work", bufs=3) as wp,
            tc.tile_pool(name="ps", bufs=1, space="PSUM") as pp,
            tc.tile_pool(name="ps2", bufs=2, space="PSUM") as pp2,
        ):
            # ---- resident constants / weights ----
            w = {}
            for k, shp in bspec.items():
                w[k] = cp.tile(shp, B16, tag=k, name=k)
                nc.sync.dma_start(w[k][:], d_w[k][:])
            for k, shp in fspec.items():
                w[k] = cp.tile(shp, F32, tag=k, name=k)
                nc.sync.dma_start(w[k][:], d_w[k][:])

            xbufs = []
            for j in range(3):
                xb = cp.tile([INPUT_DIM + 1, B], B16, tag=f"xb{j}", name=f"xb{j}")
                nc.vector.memset(xb[INPUT_DIM:, :], 1.0)
                xbufs.append(xb)
            mbufs = [cp.tile([128, B], B16, tag=f"mb{j}", name=f"mb{j}")
                     for j in range(3)]

            # state lives in bf16 (the bf16 matmul-input rounding dominates)
            ym = [cp.tile([L, B], B16, tag=f"ym{i}", name=f"ym{i}") for i in range(2)]
            ys = [cp.tile([L, B], B16, tag=f"ys{i}", name=f"ys{i}") for i in range(2)]
            nc.vector.memset(ys[0][:], 0.0)

            mm = nc.tensor.matmul

            # Warm the PE's clock past every weight DMA with K=1 dummy
            # matmuls so steady-state matmuls only wait on one producer.
            psDEF = pp.tile([128, 2 * B], F32, tag="psDEF", name="psDEF_warm")
            for k in bspec:
                mm(psDEF[0:1, 0:1], w[k][0:1, 0:1], w[k][0:1, 1:2],
                   start=True, stop=True)
            # DVE/ACT read fp32 DMA-produced columns: warm those clocks too
            nf = len(fspec)
            warm_dv = cp.tile([1, 2 * nf], F32, tag="warm_dv", name="warm_dv")
            for j, k in enumerate(fspec):
                nc.vector.tensor_copy(warm_dv[0:1, j:j + 1], w[k][0:1, 0:1])
                nc.scalar.copy(warm_dv[0:1, nf + j:nf + j + 1], w[k][0:1, 0:1])

            # ---- the recurrence ----
            # Emission order == scheduler priority: within each engine, ops
            # are emitted in intended steady-state execution order.  The
            # kx (x-part) matmuls of step t+1 are emitted inside step t's
            # two PE stall windows (h_ode tanh, yode DVE) -- psAr/psAu/psC
            # are double-buffered so the target bank is free by then.
            from concourse.tile_rust import add_dep_helper
            prev_tail = None
            psB = None          # ode1 hidden pre-act of the CURRENT step
            psA = {}            # psAr/psAu/psC of the current step
            psA_n = {}          # ... of the next step (kx pre-accumulated)

            def kx_mms(t, nets):
                """x-part matmuls of step t into fresh double-buffered banks."""
                xbt = xbufs[t % 3]
                out = {}
                for net in nets:
                    ps = pp2.tile([128, 2 * B], F32, tag="ps" + net[0].upper(),
                                  name=f"ps{net[0]}{t}")
                    for m in range(2):
                        ms = slice(m * 128, (m + 1) * 128)
                        mm(ps[:, m * B:(m + 1) * B], w[net + "_kx"][:, ms],
                           xbt[:], start=(m == 0), stop=False)
                    out["ps" + net[0].upper()] = ps
                return out

            for t in range(n_tp):
                first, last = t == 0, t == n_tp - 1
                cur, nxt = t % 2, (t + 1) % 2
                mb = mbufs[t % 3]

                if first:
                    nc.sync.dma_start(xbufs[0][:INPUT_DIM, :], d_x[0])
                    nc.sync.dma_start(mbufs[0][:], d_m[0])
                    nc.sync.dma_start(xbufs[1][:INPUT_DIM, :], d_x[1])
                    nc.sync.dma_start(mbufs[1][:], d_m[1])
                    psA = kx_mms(0, ("rg1", "ug1", "ns1"))
                if t + 2 < n_tp:
                    nc.sync.dma_start(xbufs[(t + 2) % 3][:INPUT_DIM, :], d_x[t + 2])
                    nc.sync.dma_start(mbufs[(t + 2) % 3][:], d_m[t + 2])

                psAr, psAu, psC = psA["psR"], psA["psU"], psA["psN"]

                # ---- ODE update ----
                yode = wp.tile([L, B], B16, tag="yode", name="yode")
                if first:
                    # state is zero: yode(0) = dt0*ode_f(0), a host constant
                    nc.vector.tensor_scalar(yode[:], ys[0][:],
                                            w["yode0_c"][:, 0:1], None, op0=OP.add)
                    psDEF_t = psDEF
                    psA_n = kx_mms(1, ("rg1", "ug1", "ns1"))
                else:
                    psDEF_t = pp.tile([128, 2 * B], F32, tag="psDEF",
                                      name=f"psDEF{t}")
                    psF = psDEF_t[:, 0:B]
                    # h_ode halves carry ode1_b1 as ACT bias; ode2 weights are
                    # dt-prescaled so yode needs no extra multiply
                    h_ode = wp.tile([128, 2 * B], B16, tag="h_ode", name="h_ode")
                    nc.scalar.activation(h_ode[:, 0:B], psB[:, 0:B], AF.Tanh,
                                         bias=w["ode1_b1c0"][:, 0:1])
                    # stall window W0 (h_ode tanh): kx of step t+1, part 1
                    if not last:
                        psA_n = kx_mms(t + 1, ("rg1", "ug1"))
                    mm(psF, w["ode2_k0"][:], h_ode[:, 0:B],
                       start=True, stop=False)
                    nc.scalar.activation(h_ode[:, B:], psB[:, B:], AF.Tanh,
                                         bias=w["ode1_b1c1"][:, 0:1])
                    mm(psF, w["ode2_k1"][:], h_ode[:, B:], start=False, stop=True)
                    # Yode = (psF + dt*b2) + Ym: ONE DVE op
                    nc.vector.scalar_tensor_tensor(yode[:], psF, w["ode_b2c"][:, 0:1],
                                                   ym[cur][:], op0=OP.add, op1=OP.add)
                    # stall window W1 (yode DVE): kx of step t+1, part 2
                    if not last:
                        psA_n.update(kx_mms(t + 1, ("ns1",)))

                # deferred ys' of the previous step: emitted HERE so it sits
                # after yode in the DVE static order
                if prev_tail is not None:
                    pPpre, pT1, pQ = prev_tail
                    A2 = wp.tile([L, B], F32, tag="A2", name="A2")
                    nc.vector.tensor_tensor(A2[:], pPpre[:], pT1[:], op=OP.add)
                    nc.vector.tensor_tensor(ys[cur][:], A2[:], pQ[:], op=OP.add)
                    prev_tail = None

                # gate layer 1 state parts.  PE order is chosen so psAr m0
                # (the r-chain) completes earliest: rg_k1 m0 sits right where
                # the ys' adds land (~yode+1.3us), with ug/ode1 matmuls
                # filling the gap before it.
                if not last:
                    psBn = pp.tile([128, 2 * B], F32, tag="psB", name=f"psBn{t}")

                def l1(net, ps, kk, m, stop=False):
                    ms = slice(m * 128, (m + 1) * 128)
                    src = yode if kk == "_k0" else ys[cur]
                    return mm(ps[:, m * B:(m + 1) * B], w[net + kk][:, ms], src[:],
                              start=False, stop=stop)

                def ode1(m, src):
                    ms = slice(m * 128, (m + 1) * 128)
                    return mm(psBn[:, m * B:(m + 1) * B], w["ode1_w"][:, ms],
                              src[:], start=(m == 0), stop=False)

                l1("rg1", psAr, "_k0", 0)
                l1("ug1", psAu, "_k0", 0)
                l1("ug1", psAu, "_k0", 1, stop=first)
                l1("rg1", psAr, "_k0", 1, stop=first)
                i_k1 = None
                if not first:  # ys(t=0) == 0: skip the k1 matmuls entirely
                    l1("rg1", psAr, "_k1", 0)
                    l1("rg1", psAr, "_k1", 1, stop=True)
                    l1("ug1", psAu, "_k1", 0)
                    i_k1 = l1("ug1", psAu, "_k1", 1, stop=True)
                # ode1@yode is off-cycle (psBn only closes at pm-time): FORCE
                # it after the tanh-gating k1 matmuls (the scheduler otherwise
                # hoists it, delaying tanh_r by two matmul slots)
                if not last:
                    i_o0 = ode1(0, yode)
                    if i_k1 is not None:
                        add_dep_helper(i_o0.ins, i_k1.ins, False, "pe-order")
                    ode1(1, yode)

                # r-gate layer 2 with m-half wavefront; psD shares the
                # psDEF bank (F is consumed by yode before D is written)
                psD_u, psD_r = psDEF_t[:, 0:B], psDEF_t[:, B:]
                h_r = wp.tile([128, 2 * B], B16, tag="h_r", name="h_r")
                t_ur = wp.tile([128, 2 * B], B16, tag="t_ur", name="t_ur")
                i_hr1 = None
                for m in range(2):
                    i_hr1 = nc.scalar.activation(h_r[:, m * B:(m + 1) * B],
                                                 psAr[:, m * B:(m + 1) * B], AF.Tanh)
                    mm(psD_r, w[f"rg2_k{m}"][:], h_r[:, m * B:(m + 1) * B],
                       start=(m == 0), stop=(m == 1))
                nc.scalar.activation(t_ur[:, B:], psD_r, AF.Tanh,
                                     bias=w["rg2_bc"][:, 0:1], scale=0.5)
                h_u = wp.tile([128, 2 * B], B16, tag="h_u", name="h_u")
                i_hu = nc.scalar.activation(h_u[:], psAu[:], AF.Tanh)
                # ACT order: the off-cycle merged h_u must not run between
                # the two h_r halves (the r path is the critical cycle)
                add_dep_helper(i_hu.ins, i_hr1.ins, False, "act-order")
                for m in range(2):
                    mm(psD_u, w[f"ug2_k{m}"][:], h_u[:, m * B:(m + 1) * B],
                       start=False, stop=(m == 1))
                # sign-flipped u half: t_ur_u = tanh(-z/2 - b/2), so
                # G = m*(1-sigmoid(z)) = (t_ur_u + 1) * (m/2)
                nc.scalar.activation(t_ur[:, 0:B], psD_u, AF.Tanh,
                                     bias=w["ug2_bc"][:, 0:1], scale=-0.5)

                # reset products via the prescale trick: ns1_k0/k1 carry a
                # host-side 0.5 factor, so r.state == 0.5*(1+tanh)*state
                am = wp.tile([L, B], B16, tag="am", name="am")
                nc.vector.scalar_tensor_tensor(am[:], t_ur[:, B:], 1.0, yode[:],
                                               op0=OP.add, op1=OP.mult)
                a_s = None
                if not first:
                    a_s = wp.tile([L, B], B16, tag="a_s", name="a_s")
                    nc.vector.scalar_tensor_tensor(a_s[:], t_ur[:, B:], 1.0,
                                                   ys[cur][:],
                                                   op0=OP.add, op1=OP.mult)
                # G = (t_ur_u + 1) * mh   (m_bc carries 0.5*mask)
                g = wp.tile([L, B], B16, tag="g", name="g")
                nc.vector.scalar_tensor_tensor(g[:], t_ur[:, 0:B], 1.0, mb[:L, :],
                                               op0=OP.add, op1=OP.mult)

                # ns1 state parts, am before as per m-half
                for m in range(2):
                    sl = psC[:, m * B:(m + 1) * B]
                    ms = slice(m * 128, (m + 1) * 128)
                    mm(sl, w["ns1_k0"][:, ms], am[:], start=False,
                       stop=(first and m == 1))
                    if not first:
                        mm(sl, w["ns1_k1"][:, ms], a_s[:], start=False,
                           stop=(m == 1))

                # ns layer 2 with m-half wavefront; NM first (mean cycle),
                # neg_eye (-Yode, ready early) fills the h_ns-m1 gap.
                # psE shares the psDEF bank (D consumed by t_ur by now).
                psE_m, psE_s = psDEF_t[:, 0:B], psDEF_t[:, B:]
                h_ns = wp.tile([128, 2 * B], B16, tag="h_ns", name="h_ns")
                nc.scalar.activation(h_ns[:, 0:B], psC[:, 0:B], AF.Tanh)
                i_nm0 = mm(psE_m, w["ns2_k0"][:, 0:128], h_ns[:, 0:B],
                           start=True, stop=False)
                i_ne = mm(psE_m, w["neg_eye"][:], yode[:], start=False, stop=False)
                add_dep_helper(i_ne.ins, i_nm0.ins, False, "bank-start")
                nc.scalar.activation(h_ns[:, B:], psC[:, B:], AF.Tanh)
                mm(psE_m, w["ns2_k1"][:, 0:128], h_ns[:, B:],
                   start=False, stop=False)
                mm(psE_s, w["ns2_k0"][:, 128:], h_ns[:, 0:B],
                   start=False, stop=False)
                mm(psE_s, w["ns2_k1"][:, 128:], h_ns[:, B:],
                   start=False, stop=True)

                # mean channel: Ym' = Yode + G*(NM + bm - Yode); psE_m
                # already holds NM - Yode via neg_eye, so ONE stt + add
                pm = wp.tile([L, B], B16, tag="pm", name="pm")
                nc.vector.scalar_tensor_tensor(
                    pm[:], psE_m, w["ns2_bm"][:, 0:1], g[:],
                    op0=OP.add, op1=OP.mult)
                # Ym' rides the idle Pool engine: its only consumer is the
                # NEXT step's yode op, a full step away
                nc.gpsimd.tensor_tensor(ym[nxt][:], yode[:], pm[:], op=OP.add)
                if not last:
                    for m in range(2):
                        ms = slice(m * 128, (m + 1) * 128)
                        mm(psBn[:, m * B:(m + 1) * B], w["ode1_w"][:, ms], pm[:],
                           start=False, stop=(m == 1))
                    psB = psBn
                    psA = psA_n

                # std channel: sp(z)+1e-6 = relu(z) + ln2*u + (1+u)*2^{-u}
                # + (1e-6 - 1),  u = e^{-|z|}  (exact one-Newton log1p(e^z)).
                # Ys' = (P0 + T1) + Q: P0 = Ys + G*(rl + c - Ys) via Pool,
                # T1 = (ln2*G)*u, Q = (G*(1+u))*v, v = 2^{-u}; only Q and
                # two adds trail the exps.
                # DVE P0 path emitted BEFORE the ACT chain so rl's wait
                # anchors to the ns2 matmul sem directly (emitted after zb,
                # the legalizer re-anchors it to zb's ACT sem, ~+800ns, and
                # the whole P0a->Ppre->A2 chain slides past yode)
                rl = wp.tile([L, B], F32, tag="rl", name="rl")
                nc.vector.tensor_scalar(rl[:], psE_s, w["ns2_bs"][:, 0:1],
                                        0.0, op0=OP.add, op1=OP.max)
                sB = wp.tile([L, B], F32, tag="sB", name="sB")
                nc.vector.scalar_tensor_tensor(sB[:], rl[:], CC, ys[cur][:],
                                               op0=OP.add, op1=OP.subtract)
                # slack-tolerant fp32 product on the otherwise idle Pool
                P0a = wp.tile([L, B], F32, tag="P0a", name="P0a")
                nc.gpsimd.tensor_tensor(P0a[:], sB[:], g[:], op=OP.mult)
                zb = wp.tile([L, B], F32, tag="zb", name="zb")
                nc.scalar.activation(zb[:], psE_s, AF.Abs,
                                     bias=w["ns2_bs"][:, 0:1])
                u_e = wp.tile([L, B], B16, tag="u_e", name="u_e")
                nc.scalar.activation(u_e[:], zb[:], AF.Exp, scale=-1.0)
                v_e = wp.tile([L, B], B16, tag="v_e", name="v_e")
                nc.scalar.activation(v_e[:], u_e[:], AF.Exp, scale=-LN2)
                # ug = u*G, then T1 = ln2*ug and gw = G + ug = (1+u)*G.
                # (The one-stt (u+1)*G form measures 891ns -- the add+mult
                # stt has no 2x uop -- so TT/TS ops are faster AND unblock
                # the next yode in the DVE queue.)
                ug = wp.tile([L, B], B16, tag="ug", name="ug")
                nc.vector.tensor_tensor(ug[:], u_e[:], g[:], op=OP.mult)
                T1 = wp.tile([L, B], B16, tag="T1", name="T1")
                nc.vector.tensor_scalar(T1[:], ug[:], LN2, None, op0=OP.mult)
                gw = wp.tile([L, B], B16, tag="gw", name="gw")
                nc.vector.tensor_tensor(gw[:], g[:], ug[:], op=OP.add)
                # Q also rides the Pool (deadline: the ys' add early next step)
                Q_ = wp.tile([L, B], B16, tag="Q_", name="Q_")
                nc.gpsimd.tensor_tensor(Q_[:], gw[:], v_e[:], op=OP.mult)
                Ppre = wp.tile([L, B], F32, tag="Ppre", name="Ppre")
                nc.vector.tensor_tensor(Ppre[:], ys[cur][:], P0a[:], op=OP.add)
                prev_tail = (Ppre, T1, Q_)

            # ---- final transform ----
            fin = n_tp % 2
            if prev_tail is not None:
                pPpre, pT1, pQ = prev_tail
                A2 = wp.tile([L, B], F32, tag="A2", name="A2fin")
                nc.vector.tensor_tensor(A2[:], pPpre[:], pT1[:], op=OP.add)
                nc.vector.tensor_tensor(ys[fin][:], A2[:], pQ[:], op=OP.add)
            psB = pp.tile([128, 2 * B], F32, tag="psB", name="psB_fin")
            for m in range(2):
                sl = psB[:, m * B:(m + 1) * B]
                ms = slice(m * 128, (m + 1) * 128)
                mm(sl, w["tz1_k0"][:, ms], ym[fin][:], start=(m == 0), stop=False)
                mm(sl, w["tz1_k1"][:, ms], ys[fin][:], start=False, stop=True)
            h_tz = wp.tile([128, 2 * B], B16, tag="h_ode", name="h_tz")
            nc.scalar.activation(h_tz[:, 0:B], psB[:, 0:B], AF.Tanh,
                                 bias=w["tz1_bc0"][:, 0:1])
            nc.scalar.activation(h_tz[:, B:], psB[:, B:], AF.Tanh,
                                 bias=w["tz1_bc1"][:, 0:1])
            psE = pp.tile([128, 2 * B], F32, tag="psDEF", name="psDEF_fin")
            for m in range(2):
                sl = psE[:, m * B:(m + 1) * B]
                ms = slice(m * 128, (m + 1) * 128)
                mm(sl, w["tz2_k0"][:, ms], h_tz[:, 0:B], start=(m == 0), stop=False)
                mm(sl, w["tz2_k1"][:, ms], h_tz[:, B:], start=False, stop=True)
            o_m = wp.tile([L, B], F32, tag="o_m", name="o_m")
            nc.scalar.activation(o_m[:], psE[:, 0:B], AF.Identity,
                                 bias=w["tz2_bm"][:, 0:1])
            o_s = wp.tile([L, B], F32, tag="o_s", name="o_s")
            nc.scalar.activation(o_s[:], psE[:, B:], AF.Abs,
                                 bias=w["tz2_bs"][:, 0:1])
            nc.sync.dma_start(d_om[:], o_m[:])
            nc.sync.dma_start(d_os[:], o_s[:])

    nc.compile()
    return nc


# --------------------------------------------------------------------------
# host-side packing
# --------------------------------------------------------------------------
def _dts(obs, n_tp):
    F = np.float32
    dd = (obs[:-1] - obs[1:])[::-1]
    return np.concatenate([np.full((1,), -0.01, F), dd]).astype(F)


def _prep_in_maps(inputs, n_tp):
    F = np.float32
    d = {k: np.ascontiguousarray(np.asarray(v, F)) for k, v in inputs.items()}
    data = d["data"][:, :n_tp]
    obs = np.asarray(inputs["obs_tps"], F)[:n_tp]
    dts = _dts(obs, n_tp)
    dt1 = F(dts[1])

    # [t, c, subj], reversed in time, bf16
    x_rev = np.ascontiguousarray(data.transpose(1, 2, 0)[::-1]).astype(BF)
    # observation half-mask 0.5*m per (t, subj), broadcast to 128 partitions
    m_t = F(0.5) * (data[:, :, HALF:].sum(axis=2) > 0).astype(F)  # [subj, t]
    m_rev = m_t.T[::-1].astype(BF)                                # [t, subj]
    m_bc = np.ascontiguousarray(
        np.broadcast_to(m_rev[:, None, :], (n_tp, 128, N_SUBJ)))

    def kx(w1, b1):
        return np.vstack([w1[2 * L:], b1[None, :]])

    bf = {
        "ug1_k0": d["ug_w1"][:L], "ug1_k1": d["ug_w1"][L:2 * L],
        "ug1_kx": kx(d["ug_w1"], d["ug_b1"]),
        "rg1_k0": d["rg_w1"][:L], "rg1_k1": d["rg_w1"][L:2 * L],
        "rg1_kx": kx(d["rg_w1"], d["rg_b1"]),
        "ns1_k0": d["ns_w1"][:L] * F(0.5), "ns1_k1": d["ns_w1"][L:2 * L] * F(0.5),
        "ns1_kx": kx(d["ns_w1"], d["ns_b1"]),
        "ode1_w": d["ode_w1"],
        "neg_eye": -np.eye(L, dtype=F),
        "ode2_k0": d["ode_w2"][:128] * dt1, "ode2_k1": d["ode_w2"][128:] * dt1,
        "ug2_k0": d["ug_w2"][:128], "ug2_k1": d["ug_w2"][128:],
        "rg2_k0": d["rg_w2"][:128], "rg2_k1": d["rg_w2"][128:],
        "ns2_k0": d["ns_w2"][:128], "ns2_k1": d["ns_w2"][128:],
        "tz1_k0": d["tz_w1"][:L], "tz1_k1": d["tz_w1"][L:],
        "tz2_k0": d["tz_w2"][:128], "tz2_k1": d["tz_w2"][128:],
    }
    shared = {k: np.ascontiguousarray(v.astype(BF)) for k, v in bf.items()}
    col = lambda v: np.ascontiguousarray(v.reshape(-1, 1).astype(F))
    shared["ug2_bc"] = col(d["ug_b2"] * F(-0.5))   # sign-flipped u half
    shared["rg2_bc"] = col(d["rg_b2"] * F(0.5))
    shared["ns2_bm"] = col(d["ns_b2"][:L])
    shared["ns2_bs"] = col(d["ns_b2"][L:])
    shared["tz2_bm"] = col(d["tz_b2"][:L])
    shared["tz2_bs"] = col(d["tz_b2"][L:])
    shared["ode1_b1c0"] = col(d["ode_b1"][:128])
    shared["ode1_b1c1"] = col(d["ode_b1"][128:])
    shared["ode_b2c"] = col(d["ode_b2"] * dt1)
    shared["tz1_bc0"] = col(d["tz_b1"][:128])
    shared["tz1_bc1"] = col(d["tz_b1"][128:])
    # yode(0) = dt0 * ode_f(0) with zero initial state
    ode_f0 = np.tanh(d["ode_b1"]) @ d["ode_w2"] + d["ode_b2"]
    shared["yode0_c"] = col(ode_f0 * F(dts[0]))

    in_maps = []
    for c in range(N_CORES):
        m = dict(shared)
        m["x_rev"] = np.ascontiguousarray(x_rev[:, :, c * B:(c + 1) * B])
        m["m_bc"] = np.ascontiguousarray(m_bc[:, :, c * B:(c + 1) * B])
        in_maps.append(m)
    return in_maps


def kernel(**inputs):
    from concourse.bass_utils import run_bass_kernel_spmd

    obs = np.asarray(inputs["obs_tps"], np.float32)[:N_TP]
    dts = _dts(obs, N_TP)
    key = (N_TP, tuple(np.asarray(dts, np.float64).tolist()))
    if key not in _CACHE:
        _CACHE[key] = _build(N_TP, dts)
    nc = _CACHE[key]

    in_maps = _prep_in_maps(inputs, N_TP)
    res = run_bass_kernel_spmd(nc, in_maps, list(range(N_CORES)))
    outs = res.results

    mean = np.empty((1, N_SUBJ, L), np.float32)
    std = np.empty((1, N_SUBJ, L), np.float32)
    for c in range(N_CORES):
        mean[0, c * B:(c + 1) * B] = outs[c]["out_m"].T
        std[0, c * B:(c + 1) * B] = outs[c]["out_s"].T
    return mean, std
